# revision 16
# baseline (speedup 1.0000x reference)
"""Trainium2 Bass kernel for nn_CrossAttentionBlock (B=4, C=512, H=W=64).

Decomposition across 8 NeuronCores: core = (batch b, spatial half h), where
the query/token split is INTERLEAVED: core h owns tokens {t : 32h <= t%64 < 32h+32}.
With the faithful torch-.view semantics (y [B,N,C8] reinterpreted as
[B,C8,H,W]), view-channel c at view-spatial (h',w) reads y[token c*64+h', w],
so a core owning all tokens with t%64 in its half can reconstruct the FULL
contraction input yv[:, j] for its half of the view-spatial axis locally --
no exchange of y is needed. AdaIN consumes W_y only through per-channel
spatial statistics, which are additive over the spatial axis, so the cores
in a pair exchange tiny partial bn-statistics (x0 stats and W_y stats)
instead of the y tensor.

All matmuls run in bf16 (inputs host-cast / cast on write), fp32 PSUM
accumulation; final output math in fp32. Measured end-to-end error vs the
fp32 reference ~1e-3 max-rel, inside the 2e-2 gate.

Schedule: main loop is software-pipelined (f-matmuls run two iterations
ahead of the exp-gated y-matmuls); conv projections (stage blocks) and the
q0 transpose/normalize chain are interleaved into the PE stream; input DMA
issues are spread across engine queues; partial-stat collectives ride a
prewarmed CC stream.
"""
import numpy as np
import ml_dtypes
from contextlib import ExitStack

import concourse.bass as bass
import concourse.tile as tile
from concourse import mybir
from concourse.bass_utils import run_bass_kernel_spmd

FP32 = mybir.dt.float32
BF16 = mybir.dt.bfloat16
ALU = mybir.AluOpType
ACTF = mybir.ActivationFunctionType

B, C, H, W = 4, 512, 64, 64
N = H * W          # 4096 tokens
C8 = C // 8        # 64 inner channels
NH = N // 2        # 2048 tokens per core
HC = NH            # spatial half size (columns of out per core)
EPS = 1e-5

REPLICA_PAIRS = [[0, 1], [2, 3], [4, 5], [6, 7]]

_T0 = np.array([c * 64 + j for c in range(64) for j in range(32)])
_T1 = _T0 + 32
PERMS = [np.concatenate([_T0, _T1]), np.concatenate([_T1, _T0])]

NPBF16 = ml_dtypes.bfloat16


def _split_excess_waits(nc, max_waits=1, drain_max=1):
    """walrus rejects instructions carrying more than ~2 sync waits; move
    extras to preceding NoOps on the same engine (semantics preserved: waits
    run before the instruction, engine streams are sequential)."""
    for blk in nc.main_func.blocks:
        insts = blk.instructions
        k = 0
        while k < len(insts):
            inst = insts[k]
            si = inst.sync_info
            cap = drain_max if inst.opcode == "Drain" else max_waits
            if si is not None and si.on_wait and len(si.on_wait) > cap:
                waits = list(si.on_wait)
                keep = waits[-cap:]
                extra = waits[:-cap]
                pos = k
                for j in range(0, len(extra), cap):
                    nop = mybir.InstNoOp(name=f"{inst.name}-wsplit{j}", ins=[], outs=[])
                    nop.engine = inst.engine
                    nop.sync_info = mybir.SyncInfo(
                        on_wait=extra[j : j + cap], on_update=[]
                    )
                    insts.insert(pos, nop)
                    pos += 1
                    k += 1
                inst.sync_info = mybir.SyncInfo(on_wait=keep, on_update=list(si.on_update))
            k += 1


def build_nc():
    nc = bass.Bass()

    x1t = nc.dram_tensor("x1t", [128, 8, 4, 512], BF16, kind="ExternalInput")
    x0ht = nc.dram_tensor("x0ht", [128, 8, 4, 512], BF16, kind="ExternalInput")
    x0at = nc.dram_tensor("x0at", [128, 4, HC], FP32, kind="ExternalInput")
    tp_wT = nc.dram_tensor("tp_wT", [C, 128], BF16, kind="ExternalInput")
    tp_b = nc.dram_tensor("tp_b", [128, 1], FP32, kind="ExternalInput")
    g_wT = nc.dram_tensor("g_wT", [C, C8], BF16, kind="ExternalInput")
    g_b_bc = nc.dram_tensor("g_b_bc", [128, C8], FP32, kind="ExternalInput")
    W_wT = nc.dram_tensor("W_wT", [C8, C], BF16, kind="ExternalInput")
    W_b4 = nc.dram_tensor("W_b4", [128, 4], FP32, kind="ExternalInput")
    ident = nc.dram_tensor("ident", [C8 + 1, C8 + 1], FP32, kind="ExternalInput")
    ident16 = nc.dram_tensor("ident16", [C8, C8], BF16, kind="ExternalInput")
    out = nc.dram_tensor("out", [C, HC], FP32, kind="ExternalOutput")

    y_loc = nc.dram_tensor("y_loc", [NH, C8], BF16)
    # partial bn-stats exchange buffers: row p holds that partition's
    # [oc(4) x mb(4) x 6] stats flattened; AllGather stacks partner at 128..255
    xst_send = nc.dram_tensor("xst_send", [128, 96], FP32)
    xst_recv = nc.dram_tensor("xst_recv", [256, 96], FP32)
    wst_send = nc.dram_tensor("wst_send", [128, 96], FP32)
    wst_recv = nc.dram_tensor("wst_recv", [256, 96], FP32)

    with tile.TileContext(nc) as tc, ExitStack() as ctx:
        wpool = ctx.enter_context(tc.tile_pool(name="weights", bufs=1))
        big = ctx.enter_context(tc.tile_pool(name="big", bufs=1))

        # ---- weights to SBUF (scalar-engine queue; ACT is idle in the lead) ----
        tp_w_sb = wpool.tile([128, 4, 128], BF16)
        nc.scalar.dma_start(out=tp_w_sb[:], in_=tp_wT[:].rearrange("(c p) o -> p c o", c=4))
        g_w_sb = wpool.tile([128, 4, C8], BF16)
        nc.scalar.dma_start(out=g_w_sb[:], in_=g_wT[:].rearrange("(c p) o -> p c o", c=4))
        tp_b_sb = wpool.tile([128, 1], FP32)
        nc.scalar.dma_start(out=tp_b_sb[:], in_=tp_b[:])
        g_b_sb = wpool.tile([128, C8], FP32)
        nc.scalar.dma_start(out=g_b_sb[:], in_=g_b_bc[:])
        W_w_sb = wpool.tile([C8, C], BF16)
        nc.scalar.dma_start(out=W_w_sb[:], in_=W_wT[:])
        W_b_sb = wpool.tile([128, 4], FP32)
        nc.scalar.dma_start(out=W_b_sb[:], in_=W_b4[:])
        id_sb = wpool.tile([C8 + 1, C8 + 1], FP32)
        nc.scalar.dma_start(out=id_sb[:], in_=ident[:])
        id16_sb = wpool.tile([C8, C8], BF16)
        nc.scalar.dma_start(out=id16_sb[:], in_=ident16[:])
        # prewarm the Sqrt activation table off the critical tail
        sq_scr = wpool.tile([128, 1], FP32)
        nc.scalar.sqrt(sq_scr[:], tp_b_sb[:])

        # ---- persistent big tensors ----
        theta_sb = big.tile([C8, N], BF16)        # keys [64, 4096]
        phi_sb = big.tile([C8, NH], BF16)         # own queries [64, 2048]
        g_extT = big.tile([128, 32, C8 + 1], BF16)  # [key-chunk, mi, 65]
        yT_sb = big.tile([C8 + 1, NH], FP32)
        yv_sb = big.tile([C8, HC], BF16)          # local view input [64, 2048]
        x0a_sb = big.tile([128, 4, HC], FP32)     # x0 own spatial half
        xs_st = big.tile([128, 4, 4, 6], FP32)    # x0 partial bn stats
        ws_st = big.tile([128, 4, 4, 6], FP32)    # W_y partial bn stats
        xall_sb = big.tile([128, 2, 4, 4, 6], FP32)   # [p, replica, oc, mb, 6]
        wall_sb = big.tile([128, 2, 4, 4, 6], FP32)

        nc.gpsimd.memset(g_extT[:, :, C8:C8 + 1], 1.0)

        # main PSUM pool: ft (2 bufs x 2 banks) + py (1 buf x 2 banks) = 6 banks
        ps_main = ctx.enter_context(tc.tile_pool(name="ps_main", bufs=2, space="PSUM"))
        ppool = ctx.enter_context(tc.tile_pool(name="pT", bufs=3))

        stage_ctx = ExitStack()
        x1pool = stage_ctx.enter_context(tc.tile_pool(name="x1blk", bufs=3))
        x0pool = stage_ctx.enter_context(tc.tile_pool(name="x0blk", bufs=3))
        gstage = stage_ctx.enter_context(tc.tile_pool(name="gstage", bufs=2))
        ps_st = stage_ctx.enter_context(tc.tile_pool(name="ps_st", bufs=2, space="PSUM"))

        stage_state = {}

        def stage_a(blk):
            """x1/x0h loads, theta/phi projection, g projection [ch, tok]."""
            x1b = x1pool.tile([128, 4, 512], BF16, tag="x1")
            nc.sync.dma_start(out=x1b[:], in_=x1t[:, blk, :, :])
            x0b = x0pool.tile([128, 4, 512], BF16, tag="x0")
            nc.gpsimd.dma_start(out=x0b[:], in_=x0ht[:, blk, :, :])
            ptp = ps_st.tile([128, 512], FP32, tag="ps")
            for c in range(4):
                nc.tensor.matmul(ptp[:], tp_w_sb[:, c, :], x1b[:, c, :],
                                 start=(c == 0), stop=(c == 3))
            cols = slice(blk * 512, (blk + 1) * 512)
            nc.vector.tensor_scalar_add(theta_sb[:, cols], ptp[0:C8, :],
                                        tp_b_sb[0:C8, :])
            if blk < 4:
                nc.vector.tensor_scalar_add(phi_sb[:, cols], ptp[C8:128, :],
                                            tp_b_sb[C8:128, :])
            pg = ps_st.tile([128, 512], FP32, tag="ps")
            for c in range(4):
                nc.tensor.matmul(pg[0:C8, :], g_w_sb[:, c, :], x0b[:, c, :],
                                 start=(c == 0), stop=(c == 3))
            gcols = gstage.tile([C8, 512], FP32, tag="gc")
            nc.vector.tensor_copy(gcols[:], pg[0:C8, :])
            stage_state[blk] = gcols

        def stage_b(blk):
            """transpose g [ch, tok] -> g_extT [tok, ch] via PE."""
            gcols = stage_state.pop(blk)
            for k in range(4):
                mi = blk * 4 + k
                pgt = ps_st.tile([128, 512], FP32, tag="ps")
                nc.tensor.transpose(pgt[:, 0:C8], gcols[:, k * 128:(k + 1) * 128],
                                    id_sb[0:C8, 0:C8])
                nc.vector.tensor_copy(g_extT[:, mi, 0:C8], pgt[:, 0:C8])

        stage_a(0)
        stage_b(0)
        stage_a(1)
        stage_b(1)

        def f_mm(q, mi):
            ft = ps_main.tile([128, 1024], FP32, tag="ft")
            for s in range(2):
                nc.tensor.matmul(
                    ft[:, s * 512:(s + 1) * 512],
                    theta_sb[:, mi * 128:(mi + 1) * 128],
                    phi_sb[:, q * 1024 + s * 512: q * 1024 + (s + 1) * 512],
                    start=True, stop=True)
            return ft

        def exp_y(mi, ft, py):
            pt = ppool.tile([128, 1024], BF16, tag="pt")
            nc.scalar.activation(pt[:], ft[:], ACTF.Exp)
            for s in range(2):
                nc.tensor.matmul(
                    py[:, s * 512:(s + 1) * 512],
                    g_extT[:, mi, :],
                    pt[:, s * 512:(s + 1) * 512],
                    start=(mi == 0), stop=(mi == 31))

        def xpose_norm(j, ps_t, ystage):
            ptile = ps_t.tile([128, C8 + 1], FP32, tag="pt")
            nc.tensor.transpose(ptile[:], yT_sb[:, j * 128:(j + 1) * 128], id_sb[:])
            rec = ystage.tile([128, 1], FP32, tag="rec")
            nc.vector.reciprocal(rec[:], ptile[:, C8:C8 + 1])
            tmp = ystage.tile([128, C8], FP32, tag="tmp")
            nc.vector.tensor_scalar_mul(tmp[:], ptile[:, 0:C8], rec[:])
            yst = ystage.tile([128, C8], BF16, tag="yst")
            nc.vector.tensor_add(yst[:], tmp[:], g_b_sb[:])
            nc.sync.dma_start(out=y_loc[j * 128:(j + 1) * 128, :], in_=yst[:])

        # ---- q0 main loop (software pipelined), stages 2..7 interleaved ----
        py0 = ps_main.tile([C8 + 1, 1024], FP32, tag="py", bufs=1)
        fts = {0: f_mm(0, 0), 1: f_mm(0, 1)}
        for mi in range(32):
            exp_y(mi, fts.pop(mi), py0)
            if mi + 2 < 32:
                fts[mi + 2] = f_mm(0, mi + 2)
            blk = mi // 4 + 1
            if mi % 4 == 0 and blk + 1 <= 7:
                stage_a(blk + 1)
            if mi % 4 == 2 and blk <= 7 and blk in stage_state:
                stage_b(blk)
            if mi == 13:
                for cch in range(4):
                    nc.gpsimd.dma_start(out=x0a_sb[:, cch, :], in_=x0at[:, cch, :])
        nc.vector.tensor_copy(yT_sb[:, 0:1024], py0[:])
        stage_ctx.close()

        t1_ctx = ExitStack()
        ps_t = t1_ctx.enter_context(tc.tile_pool(name="ps_t", bufs=2, space="PSUM"))
        ystage = t1_ctx.enter_context(tc.tile_pool(name="ystage", bufs=2))

        # ---- q1 main loop; q0 transpose/normalize + x0 stats interleaved ----
        py1 = ps_main.tile([C8 + 1, 1024], FP32, tag="py", bufs=1)
        fts = {0: f_mm(1, 0), 1: f_mm(1, 1)}
        for mi in range(32):
            exp_y(mi, fts.pop(mi), py1)
            if mi + 2 < 32:
                fts[mi + 2] = f_mm(1, mi + 2)
            if mi >= 2 and mi % 2 == 0 and (mi - 2) // 2 <= 7:
                xpose_norm((mi - 2) // 2, ps_t, ystage)
            if mi == 19:
                nc.sync.dma_start(
                    out=yv_sb[0:32, :],
                    in_=y_loc[0:1024, :].rearrange("(a b) w -> a (b w)", a=32))
            if mi == 21:
                for oc in range(4):
                    for mb in range(4):
                        nc.vector.bn_stats(xs_st[:, oc, mb, :],
                                           x0a_sb[:, oc, mb * 512:(mb + 1) * 512])
                nc.sync.dma_start(out=xst_send[:], in_=xs_st[:])
                nc.gpsimd.collective_compute(
                    "AllGather", ALU.bypass,
                    replica_groups=REPLICA_PAIRS,
                    ins=[xst_send[:]],
                    outs=[xst_recv[:]],
                )
                nc.sync.dma_start(
                    out=xall_sb[:],
                    in_=xst_recv[:].rearrange("(w p) g -> p w g", w=2))

        # ---- tail: pipelined yT copy + transpose/normalize for q1 ----
        for j8 in range(8):
            nc.vector.tensor_copy(
                yT_sb[:, 1024 + j8 * 128: 1024 + (j8 + 1) * 128],
                py1[:, j8 * 128:(j8 + 1) * 128])
            xpose_norm(8 + j8, ps_t, ystage)
        nc.sync.dma_start(
            out=yv_sb[32:64, :],
            in_=y_loc[1024:2048, :].rearrange("(a b) w -> a (b w)", a=32))
        t1_ctx.close()

        tail_ctx = ExitStack()
        ps_W = tail_ctx.enter_context(tc.tile_pool(name="ps_W", bufs=2, space="PSUM"))
        sc = tail_ctx.enter_context(tc.tile_pool(name="sc", bufs=1))
        outp = tail_ctx.enter_context(tc.tile_pool(name="outp", bufs=2))

        # ---- W_y partial stats ----
        for oc in range(4):
            for mb in range(4):
                pw = ps_W.tile([128, 512], FP32, tag="pw")
                nc.tensor.matmul(pw[:], W_w_sb[:, oc * 128:(oc + 1) * 128],
                                 yv_sb[:, mb * 512:(mb + 1) * 512],
                                 start=True, stop=True)
                nc.vector.bn_stats(ws_st[:, oc, mb, :], pw[:])
        nc.sync.dma_start(out=wst_send[:], in_=ws_st[:])
        nc.gpsimd.collective_compute(
            "AllGather", ALU.bypass,
            replica_groups=REPLICA_PAIRS,
            ins=[wst_send[:]],
            outs=[wst_recv[:]],
        )
        nc.sync.dma_start(
            out=wall_sb[:],
            in_=wst_recv[:].rearrange("(w p) g -> p w g", w=2))

        # ---- combine stats ----
        xagg = sc.tile([128, 4, 2], FP32, tag="xagg")
        wagg = sc.tile([128, 4, 2], FP32, tag="wagg")
        for oc in range(4):
            nc.vector.bn_aggr(xagg[:, oc, :], xall_sb[:, :, oc])
            nc.vector.bn_aggr(wagg[:, oc, :], wall_sb[:, :, oc])

        vc = sc.tile([128, 4], FP32, tag="vc")
        nc.vector.tensor_scalar_add(vc[:], xagg[:, :, 1], EPS)
        rc = sc.tile([128, 4], FP32, tag="rc")
        nc.vector.reciprocal(rc[:], vc[:])
        vs = sc.tile([128, 4], FP32, tag="vs")
        nc.vector.tensor_scalar_add(vs[:], wagg[:, :, 1], EPS)
        ratio = sc.tile([128, 4], FP32, tag="ratio")
        nc.vector.tensor_mul(ratio[:], vs[:], rc[:])
        rr = sc.tile([128, 4], FP32, tag="rr")
        nc.scalar.sqrt(rr[:], ratio[:])
        mus = sc.tile([128, 4], FP32, tag="mus")
        nc.vector.tensor_add(mus[:], wagg[:, :, 0], W_b_sb[:])
        rmc = sc.tile([128, 4], FP32, tag="rmc")
        nc.vector.tensor_mul(rmc[:], rr[:], xagg[:, :, 0])
        tt = sc.tile([128, 4], FP32, tag="tt")
        nc.vector.tensor_sub(tt[:], mus[:], rmc[:])

        # ---- per-channel affine, split across DVE / ACT / Pool ----
        out_dma_engines = [nc.sync, nc.gpsimd, nc.scalar, nc.sync]
        half_engines = [nc.vector, nc.scalar, nc.gpsimd, nc.vector,
                        nc.scalar, nc.vector, nc.scalar, nc.gpsimd]
        for oc in range(4):
            ot = outp.tile([128, HC], FP32, tag="ot")
            for hv in range(2):
                eng = half_engines[oc * 2 + hv]
                colz = slice(hv * 1024, (hv + 1) * 1024)
                if eng is nc.scalar:
                    nc.scalar.activation(ot[:, colz], x0a_sb[:, oc, colz],
                                         ACTF.Identity,
                                         bias=tt[:, oc:oc + 1],
                                         scale=rr[:, oc:oc + 1])
                else:
                    eng.tensor_scalar(ot[:, colz], x0a_sb[:, oc, colz],
                                      rr[:, oc:oc + 1], tt[:, oc:oc + 1],
                                      ALU.mult, ALU.add)
            out_dma_engines[oc].dma_start(out=out[oc * 128:(oc + 1) * 128, :],
                                          in_=ot[:])

        tail_ctx.close()

    _split_excess_waits(nc)
    return nc


_NC_CACHE = None


def _get_nc():
    global _NC_CACHE
    if _NC_CACHE is None:
        _NC_CACHE = build_nc()
    return _NC_CACHE


def _prep_weights(g_w, g_b, theta_w, theta_b, phi_w, phi_b, W_w, W_b):
    tp_wT = np.ascontiguousarray(
        np.concatenate([theta_w, phi_w], axis=0).T).astype(NPBF16)
    tp_b = np.ascontiguousarray(
        np.concatenate([theta_b, phi_b]).astype(np.float32)[:, None])
    g_wT = np.ascontiguousarray(np.asarray(g_w, np.float32).T).astype(NPBF16)
    g_b_bc = np.ascontiguousarray(
        np.broadcast_to(np.asarray(g_b, np.float32), (128, C8)))
    W_wT = np.ascontiguousarray(np.asarray(W_w, np.float32).T).astype(NPBF16)
    W_b4 = np.ascontiguousarray(
        np.asarray(W_b, np.float32).reshape(4, 128).T)
    ident = np.eye(C8 + 1, dtype=np.float32)
    ident16 = np.eye(C8, dtype=np.float32).astype(NPBF16)
    return tp_wT, tp_b, g_wT, g_b_bc, W_wT, W_b4, ident, ident16


def _core_inputs(x0f, x1f, weights, core):
    tp_wT, tp_b, g_wT, g_b_bc, W_wT, W_b4, ident, ident16 = weights
    b, h = core // 2, core % 2
    perm = PERMS[h]
    x1p = x1f[b][:, perm]
    x0hp = x0f[b][:, perm]
    x0aff = x0f[b][:, h * HC:(h + 1) * HC]
    x1t = np.ascontiguousarray(
        x1p.reshape(4, 128, 8, 512).transpose(1, 2, 0, 3)).astype(NPBF16)
    x0ht = np.ascontiguousarray(
        x0hp.reshape(4, 128, 8, 512).transpose(1, 2, 0, 3)).astype(NPBF16)
    x0at = np.ascontiguousarray(
        x0aff.reshape(4, 128, HC).transpose(1, 0, 2))
    return {
        "x1t": x1t,
        "x0ht": x0ht,
        "x0at": x0at,
        "tp_wT": tp_wT,
        "tp_b": tp_b,
        "g_wT": g_wT,
        "g_b_bc": g_b_bc,
        "W_wT": W_wT,
        "W_b4": W_b4,
        "ident": ident,
        "ident16": ident16,
    }


def kernel(x0, x1, g_w, g_b, theta_w, theta_b, phi_w, phi_b, W_w, W_b):
    x0 = np.asarray(x0, dtype=np.float32)
    x1 = np.asarray(x1, dtype=np.float32)
    x0f = x0.reshape(B, C, N)
    x1f = x1.reshape(B, C, N)
    weights = _prep_weights(g_w, g_b, theta_w, theta_b, phi_w, phi_b, W_w, W_b)

    in_maps = [_core_inputs(x0f, x1f, weights, core) for core in range(8)]
    nc = _get_nc()
    res = run_bass_kernel_spmd(nc, in_maps, core_ids=list(range(8)))

    out = np.empty((B, C, N), dtype=np.float32)
    for core in range(8):
        b, h = core // 2, core % 2
        out[b][:, h * HC:(h + 1) * HC] = res.results[core]["out"]
    return out.reshape(B, C, H, W)


# revision 24
# speedup vs baseline: 1.0746x; 1.0746x over previous
"""Trainium2 Bass kernel for nn_CrossAttentionBlock (B=4, C=512, H=W=64).

Decomposition across 8 NeuronCores: core = (batch b, spatial half h), where
the query/token split is INTERLEAVED: core h owns tokens {t : 32h <= t%64 < 32h+32}.
With the faithful torch-.view semantics (y [B,N,C8] reinterpreted as
[B,C8,H,W]), view-channel c at view-spatial (h',w) reads y[token c*64+h', w],
so a core owning all tokens with t%64 in its half can reconstruct the FULL
contraction input yv[:, j] for its half of the view-spatial axis locally --
no exchange of y is needed. AdaIN consumes W_y only through per-channel
spatial statistics, which are additive over the spatial axis, so the cores
in a pair exchange tiny partial bn-statistics (x0 stats and W_y stats)
instead of the y tensor.

All matmuls run in bf16 (inputs host-cast / cast on write), fp32 PSUM
accumulation; final output math in fp32. Measured end-to-end error vs the
fp32 reference ~1e-3 max-rel, inside the 2e-2 gate.

Schedule: main loop is software-pipelined (f-matmuls run two iterations
ahead of the exp-gated y-matmuls); conv projections (stage blocks) and the
q0 transpose/normalize chain are interleaved into the PE stream; input DMA
issues are spread across engine queues; partial-stat collectives ride a
prewarmed CC stream.
"""
import numpy as np
import ml_dtypes
from contextlib import ExitStack

import concourse.bass as bass
import concourse.tile as tile
from concourse import mybir
from concourse.bass_utils import run_bass_kernel_spmd

FP32 = mybir.dt.float32
BF16 = mybir.dt.bfloat16
ALU = mybir.AluOpType
ACTF = mybir.ActivationFunctionType

B, C, H, W = 4, 512, 64, 64
N = H * W          # 4096 tokens
C8 = C // 8        # 64 inner channels
NH = N // 2        # 2048 tokens per core
HC = NH            # spatial half size (columns of out per core)
EPS = 1e-5

REPLICA_PAIRS = [[0, 1], [2, 3], [4, 5], [6, 7]]

_T0 = np.array([c * 64 + j for c in range(64) for j in range(32)])
_T1 = _T0 + 32
PERMS = [np.concatenate([_T0, _T1]), np.concatenate([_T1, _T0])]

NPBF16 = ml_dtypes.bfloat16


def _split_excess_waits(nc, max_waits=1, drain_max=1):
    """walrus rejects instructions carrying more than ~2 sync waits; move
    extras to preceding NoOps on the same engine (semantics preserved: waits
    run before the instruction, engine streams are sequential)."""
    for blk in nc.main_func.blocks:
        insts = blk.instructions
        k = 0
        while k < len(insts):
            inst = insts[k]
            si = inst.sync_info
            cap = drain_max if inst.opcode == "Drain" else max_waits
            if si is not None and si.on_wait and len(si.on_wait) > cap:
                waits = list(si.on_wait)
                keep = waits[-cap:]
                extra = waits[:-cap]
                pos = k
                for j in range(0, len(extra), cap):
                    nop = mybir.InstNoOp(name=f"{inst.name}-wsplit{j}", ins=[], outs=[])
                    nop.engine = inst.engine
                    nop.sync_info = mybir.SyncInfo(
                        on_wait=extra[j : j + cap], on_update=[]
                    )
                    insts.insert(pos, nop)
                    pos += 1
                    k += 1
                inst.sync_info = mybir.SyncInfo(on_wait=keep, on_update=list(si.on_update))
            k += 1


def build_nc():
    nc = bass.Bass()

    x1t = nc.dram_tensor("x1t", [128, 8, 4, 512], BF16, kind="ExternalInput")
    x0ht = nc.dram_tensor("x0ht", [128, 8, 4, 512], BF16, kind="ExternalInput")
    x0at = nc.dram_tensor("x0at", [128, 4, HC], FP32, kind="ExternalInput")
    tp_wt = nc.dram_tensor("tp_wt", [128, 4, 128], BF16, kind="ExternalInput")
    tp_b = nc.dram_tensor("tp_b", [128, 1], FP32, kind="ExternalInput")
    g_wt = nc.dram_tensor("g_wt", [128, 4, C8], BF16, kind="ExternalInput")
    W_wT = nc.dram_tensor("W_wT", [C8, C], BF16, kind="ExternalInput")
    W_b4 = nc.dram_tensor("W_b4", [128, 4], FP32, kind="ExternalInput")
    ident = nc.dram_tensor("ident", [C8 + 1, C8 + 1], FP32, kind="ExternalInput")
    out = nc.dram_tensor("out", [C, HC], FP32, kind="ExternalOutput")

    y_loc = nc.dram_tensor("y_loc", [NH, C8], BF16)
    # partial bn-stats exchange buffers: row p holds that partition's
    # [oc(4) x mb(4) x 6] stats flattened; AllGather stacks partner at 128..255
    xst_send = nc.dram_tensor("xst_send", [128, 96], FP32)
    xst_recv = nc.dram_tensor("xst_recv", [256, 96], FP32)
    wst_send = nc.dram_tensor("wst_send", [128, 96], FP32)
    wst_recv = nc.dram_tensor("wst_recv", [256, 96], FP32)

    with tile.TileContext(nc) as tc, ExitStack() as ctx:
        wpool = ctx.enter_context(tc.tile_pool(name="weights", bufs=1))
        big = ctx.enter_context(tc.tile_pool(name="big", bufs=1))

        # ---- weights to SBUF; critical ones (tp_w) first on the sync queue,
        # the rest trickle on the scalar queue while stage 0 runs ----
        tp_w_sb = wpool.tile([128, 4, 128], BF16)
        nc.sync.dma_start(out=tp_w_sb[:], in_=tp_wt[:])
        g_w_sb = wpool.tile([128, 4, C8], BF16)
        nc.gpsimd.dma_start(out=g_w_sb[:], in_=g_wt[:])
        tp_b_sb = wpool.tile([128, 1], FP32)
        nc.scalar.dma_start(out=tp_b_sb[:], in_=tp_b[:])
        W_w_sb = wpool.tile([C8, C], BF16)
        nc.scalar.dma_start(out=W_w_sb[:], in_=W_wT[:])
        W_b_sb = wpool.tile([128, 4], FP32)
        nc.scalar.dma_start(out=W_b_sb[:], in_=W_b4[:])
        id_sb = wpool.tile([C8 + 1, C8 + 1], FP32)
        nc.scalar.dma_start(out=id_sb[:], in_=ident[:])

        # ---- persistent big tensors ----
        theta_sb = big.tile([C8, N], BF16)        # keys [64, 4096]
        phi_sb = big.tile([C8, NH], BF16)         # own queries [64, 2048]
        g_extT = big.tile([128, 32, C8 + 1], BF16)  # [key-chunk, mi, 65]
        yT_sb = big.tile([C8 + 1, NH], FP32)
        yv_sb = big.tile([C8, HC], BF16)          # local view input [64, 2048]
        x0a_sb = big.tile([128, 4, HC], FP32)     # x0 own spatial half
        xs_st = big.tile([128, 4, 4, 6], FP32)    # x0 partial bn stats
        ws_st = big.tile([128, 4, 4, 6], FP32)    # W_y partial bn stats
        xall_sb = big.tile([128, 2, 4, 4, 6], FP32)   # [p, replica, oc, mb, 6]
        wall_sb = big.tile([128, 2, 4, 4, 6], FP32)

        nc.gpsimd.memset(g_extT[:, :, C8:C8 + 1], 1.0)

        # main PSUM pool: ft (2 bufs x 2 banks) + py (1 buf x 2 banks) = 6 banks
        ps_main = ctx.enter_context(tc.tile_pool(name="ps_main", bufs=2, space="PSUM"))
        ppool = ctx.enter_context(tc.tile_pool(name="pT", bufs=3))

        stage_ctx = ExitStack()
        x1pool = stage_ctx.enter_context(tc.tile_pool(name="x1blk", bufs=3))
        x0pool = stage_ctx.enter_context(tc.tile_pool(name="x0blk", bufs=3))
        gstage = stage_ctx.enter_context(tc.tile_pool(name="gstage", bufs=2))
        ps_st = stage_ctx.enter_context(tc.tile_pool(name="ps_st", bufs=2, space="PSUM"))

        stage_state = {}

        def stage_a(blk):
            """x1/x0h loads, theta/phi projection, g projection [ch, tok]."""
            x1b = x1pool.tile([128, 4, 512], BF16, tag="x1")
            nc.sync.dma_start(out=x1b[:], in_=x1t[:, blk, :, :])
            x0b = x0pool.tile([128, 4, 512], BF16, tag="x0")
            nc.gpsimd.dma_start(out=x0b[:], in_=x0ht[:, blk, :, :])
            ptp = ps_st.tile([128, 512], FP32, tag="ps")
            for c in range(4):
                nc.tensor.matmul(ptp[:], tp_w_sb[:, c, :], x1b[:, c, :],
                                 start=(c == 0), stop=(c == 3))
            cols = slice(blk * 512, (blk + 1) * 512)
            nc.vector.tensor_scalar_add(theta_sb[:, cols], ptp[0:C8, :],
                                        tp_b_sb[0:C8, :])
            if blk < 4:
                nc.vector.tensor_scalar_add(phi_sb[:, cols], ptp[C8:128, :],
                                            tp_b_sb[C8:128, :])
            pg = ps_st.tile([128, 512], FP32, tag="ps")
            for c in range(4):
                nc.tensor.matmul(pg[0:C8, :], g_w_sb[:, c, :], x0b[:, c, :],
                                 start=(c == 0), stop=(c == 3))
            gcols = gstage.tile([C8, 512], FP32, tag="gc")
            nc.vector.tensor_copy(gcols[:], pg[0:C8, :])
            stage_state[blk] = gcols

        def stage_b(blk):
            """transpose g [ch, tok] -> g_extT [tok, ch] via PE."""
            gcols = stage_state.pop(blk)
            for k in range(4):
                mi = blk * 4 + k
                pgt = ps_st.tile([128, 512], FP32, tag="ps")
                nc.tensor.transpose(pgt[:, 0:C8], gcols[:, k * 128:(k + 1) * 128],
                                    id_sb[0:C8, 0:C8])
                nc.vector.tensor_copy(g_extT[:, mi, 0:C8], pgt[:, 0:C8])

        stage_a(0)
        stage_b(0)
        stage_a(1)
        stage_b(1)

        def f_mm(q, mi):
            ft = ps_main.tile([128, 1024], FP32, tag="ft")
            for s in range(2):
                nc.tensor.matmul(
                    ft[:, s * 512:(s + 1) * 512],
                    theta_sb[:, mi * 128:(mi + 1) * 128],
                    phi_sb[:, q * 1024 + s * 512: q * 1024 + (s + 1) * 512],
                    start=True, stop=True)
            return ft

        def exp_y(mi, ft, py):
            pt = ppool.tile([128, 1024], BF16, tag="pt")
            nc.scalar.activation(pt[:], ft[:], ACTF.Exp)
            for s in range(2):
                nc.tensor.matmul(
                    py[:, s * 512:(s + 1) * 512],
                    g_extT[:, mi, :],
                    pt[:, s * 512:(s + 1) * 512],
                    start=(mi == 0), stop=(mi == 31))

        # g_b is folded into W_b on the host (it only shifts the W_y mean),
        # so normalization is reciprocal + one scaled copy per chunk. Chunks
        # are staged in groups of 4 so the DRAM bounce costs 2 DMA issues.
        def xpose_norm(j, ps_t, ystage):
            ptile = ps_t.tile([128, C8 + 1], FP32, tag="pt")
            nc.tensor.transpose(ptile[:], yT_sb[:, j * 128:(j + 1) * 128], id_sb[:])
            rec = ystage.tile([128, 1], FP32, tag="rec")
            nc.vector.reciprocal(rec[:], ptile[:, C8:C8 + 1])
            jj = j % 4
            if jj == 0:
                ystage.cur_yst4 = ystage.tile([128, 4, C8], BF16, tag="yst4")
            yst4 = ystage.cur_yst4
            nc.vector.tensor_scalar_mul(yst4[:, jj, :], ptile[:, 0:C8], rec[:])
            if jj == 3:
                j0 = j - 3
                nc.sync.dma_start(
                    out=y_loc[j0 * 128:(j0 + 4) * 128, :].rearrange(
                        "(jj p) w -> p jj w", jj=4),
                    in_=yst4[:])

        # ---- q0 main loop (software pipelined), stages 2..7 interleaved ----
        py0 = ps_main.tile([C8 + 1, 1024], FP32, tag="py", bufs=1)
        fts = {0: f_mm(0, 0), 1: f_mm(0, 1)}
        for mi in range(32):
            exp_y(mi, fts.pop(mi), py0)
            if mi + 2 < 32:
                fts[mi + 2] = f_mm(0, mi + 2)
            blk = mi // 4 + 1
            if mi % 4 == 0 and blk + 1 <= 7:
                stage_a(blk + 1)
            if mi % 4 == 2 and blk <= 7 and blk in stage_state:
                stage_b(blk)
            if mi == 13:
                for cch in range(4):
                    nc.gpsimd.dma_start(out=x0a_sb[:, cch, :], in_=x0at[:, cch, :])
        nc.vector.tensor_copy(yT_sb[:, 0:1024], py0[:])
        stage_ctx.close()

        t1_ctx = ExitStack()
        ps_t = t1_ctx.enter_context(tc.tile_pool(name="ps_t", bufs=2, space="PSUM"))
        ystage = t1_ctx.enter_context(tc.tile_pool(name="ystage", bufs=2))

        # ---- q1 main loop; q0 transpose/normalize + x0 stats interleaved ----
        py1 = ps_main.tile([C8 + 1, 1024], FP32, tag="py", bufs=1)
        fts = {0: f_mm(1, 0), 1: f_mm(1, 1)}
        for mi in range(32):
            exp_y(mi, fts.pop(mi), py1)
            if mi + 2 < 32:
                fts[mi + 2] = f_mm(1, mi + 2)
            if mi >= 2 and mi % 2 == 0 and (mi - 2) // 2 <= 7:
                xpose_norm((mi - 2) // 2, ps_t, ystage)
            if mi == 19:
                nc.sync.dma_start(
                    out=yv_sb[0:32, :],
                    in_=y_loc[0:1024, :].rearrange("(a b) w -> a (b w)", a=32))
            if mi == 21:
                for oc in range(4):
                    for mb in range(4):
                        nc.vector.bn_stats(xs_st[:, oc, mb, :],
                                           x0a_sb[:, oc, mb * 512:(mb + 1) * 512])
                nc.sync.dma_start(out=xst_send[:], in_=xs_st[:])
                nc.gpsimd.collective_compute(
                    "AllGather", ALU.bypass,
                    replica_groups=REPLICA_PAIRS,
                    ins=[xst_send[:]],
                    outs=[xst_recv[:]],
                )
                nc.sync.dma_start(
                    out=xall_sb[:],
                    in_=xst_recv[:].rearrange("(w p) g -> p w g", w=2))

        # ---- tail: pipelined yT copy + transpose/normalize for q1 ----
        for j8 in range(8):
            nc.vector.tensor_copy(
                yT_sb[:, 1024 + j8 * 128: 1024 + (j8 + 1) * 128],
                py1[:, j8 * 128:(j8 + 1) * 128])
            xpose_norm(8 + j8, ps_t, ystage)
        nc.sync.dma_start(
            out=yv_sb[32:64, :],
            in_=y_loc[1024:2048, :].rearrange("(a b) w -> a (b w)", a=32))
        t1_ctx.close()

        tail_ctx = ExitStack()
        ps_W = tail_ctx.enter_context(tc.tile_pool(name="ps_W", bufs=2, space="PSUM"))
        sc = tail_ctx.enter_context(tc.tile_pool(name="sc", bufs=1))
        outp = tail_ctx.enter_context(tc.tile_pool(name="outp", bufs=4))

        # ---- W_y partial stats ----
        for oc in range(4):
            for mb in range(4):
                pw = ps_W.tile([128, 512], FP32, tag="pw")
                nc.tensor.matmul(pw[:], W_w_sb[:, oc * 128:(oc + 1) * 128],
                                 yv_sb[:, mb * 512:(mb + 1) * 512],
                                 start=True, stop=True)
                nc.vector.bn_stats(ws_st[:, oc, mb, :], pw[:])
        nc.sync.dma_start(out=wst_send[:], in_=ws_st[:])
        nc.gpsimd.collective_compute(
            "AllGather", ALU.bypass,
            replica_groups=REPLICA_PAIRS,
            ins=[wst_send[:]],
            outs=[wst_recv[:]],
        )
        nc.gpsimd.dma_start(
            out=wall_sb[:],
            in_=wst_recv[:].rearrange("(w p) g -> p w g", w=2))

        # ---- combine stats ----
        xagg = sc.tile([128, 4, 2], FP32, tag="xagg")
        wagg = sc.tile([128, 4, 2], FP32, tag="wagg")
        for oc in range(4):
            nc.vector.bn_aggr(xagg[:, oc, :], xall_sb[:, :, oc])
            nc.vector.bn_aggr(wagg[:, oc, :], wall_sb[:, :, oc])

        vc = sc.tile([128, 4], FP32, tag="vc")
        nc.vector.tensor_scalar_add(vc[:], xagg[:, :, 1], EPS)
        rc = sc.tile([128, 4], FP32, tag="rc")
        nc.vector.reciprocal(rc[:], vc[:])
        vs = sc.tile([128, 4], FP32, tag="vs")
        nc.vector.tensor_scalar_add(vs[:], wagg[:, :, 1], EPS)
        ratio = sc.tile([128, 4], FP32, tag="ratio")
        nc.vector.tensor_mul(ratio[:], vs[:], rc[:])
        rr = sc.tile([128, 4], FP32, tag="rr")
        nc.scalar.sqrt(rr[:], ratio[:])
        mus = sc.tile([128, 4], FP32, tag="mus")
        nc.vector.tensor_add(mus[:], wagg[:, :, 0], W_b_sb[:])
        rmc = sc.tile([128, 4], FP32, tag="rmc")
        nc.vector.tensor_mul(rmc[:], rr[:], xagg[:, :, 0])
        tt = sc.tile([128, 4], FP32, tag="tt")
        nc.vector.tensor_sub(tt[:], mus[:], rmc[:])

        # ---- per-channel affine, split across DVE / ACT / Pool ----
        out_dma_engines = [nc.sync, nc.gpsimd, nc.scalar, nc.sync]
        half_engines = [nc.vector, nc.scalar, nc.gpsimd, nc.vector,
                        nc.scalar, nc.vector, nc.scalar, nc.gpsimd]
        for oc in range(4):
            ot = outp.tile([128, HC], FP32, tag="ot")
            for hv in range(2):
                eng = half_engines[oc * 2 + hv]
                colz = slice(hv * 1024, (hv + 1) * 1024)
                if eng is nc.scalar:
                    nc.scalar.activation(ot[:, colz], x0a_sb[:, oc, colz],
                                         ACTF.Identity,
                                         bias=tt[:, oc:oc + 1],
                                         scale=rr[:, oc:oc + 1])
                else:
                    eng.tensor_scalar(ot[:, colz], x0a_sb[:, oc, colz],
                                      rr[:, oc:oc + 1], tt[:, oc:oc + 1],
                                      ALU.mult, ALU.add)
            out_dma_engines[oc].dma_start(out=out[oc * 128:(oc + 1) * 128, :],
                                          in_=ot[:])

        tail_ctx.close()

    _split_excess_waits(nc)
    return nc


_NC_CACHE = None


def _get_nc():
    global _NC_CACHE
    if _NC_CACHE is None:
        _NC_CACHE = build_nc()
    return _NC_CACHE


def _prep_weights(g_w, g_b, theta_w, theta_b, phi_w, phi_b, W_w, W_b):
    tp_wT = np.ascontiguousarray(
        np.concatenate([theta_w, phi_w], axis=0).T.astype(np.float32))
    tp_wt = np.ascontiguousarray(
        tp_wT.reshape(4, 128, 128).transpose(1, 0, 2)).astype(NPBF16)
    tp_b = np.ascontiguousarray(
        np.concatenate([theta_b, phi_b]).astype(np.float32)[:, None])
    g_wT = np.asarray(g_w, np.float32).T
    g_wt = np.ascontiguousarray(
        g_wT.reshape(4, 128, C8).transpose(1, 0, 2)).astype(NPBF16)
    W_wT = np.ascontiguousarray(np.asarray(W_w, np.float32).T).astype(NPBF16)
    # g_b only shifts the W_y mean: fold W_w @ g_b into the W bias
    W_b_eff = (np.asarray(W_b, np.float32)
               + np.asarray(W_w, np.float32) @ np.asarray(g_b, np.float32))
    W_b4 = np.ascontiguousarray(W_b_eff.reshape(4, 128).T)
    ident = np.eye(C8 + 1, dtype=np.float32)
    return tp_wt, tp_b, g_wt, W_wT, W_b4, ident


def _core_inputs(x0f, x1f, weights, core):
    tp_wt, tp_b, g_wt, W_wT, W_b4, ident = weights
    b, h = core // 2, core % 2
    perm = PERMS[h]
    x1p = x1f[b][:, perm]
    x0hp = x0f[b][:, perm]
    x0aff = x0f[b][:, h * HC:(h + 1) * HC]
    x1t = np.ascontiguousarray(
        x1p.reshape(4, 128, 8, 512).transpose(1, 2, 0, 3)).astype(NPBF16)
    x0ht = np.ascontiguousarray(
        x0hp.reshape(4, 128, 8, 512).transpose(1, 2, 0, 3)).astype(NPBF16)
    x0at = np.ascontiguousarray(
        x0aff.reshape(4, 128, HC).transpose(1, 0, 2))
    return {
        "x1t": x1t,
        "x0ht": x0ht,
        "x0at": x0at,
        "tp_wt": tp_wt,
        "tp_b": tp_b,
        "g_wt": g_wt,
        "W_wT": W_wT,
        "W_b4": W_b4,
        "ident": ident,
    }


def kernel(x0, x1, g_w, g_b, theta_w, theta_b, phi_w, phi_b, W_w, W_b):
    x0 = np.asarray(x0, dtype=np.float32)
    x1 = np.asarray(x1, dtype=np.float32)
    x0f = x0.reshape(B, C, N)
    x1f = x1.reshape(B, C, N)
    weights = _prep_weights(g_w, g_b, theta_w, theta_b, phi_w, phi_b, W_w, W_b)

    in_maps = [_core_inputs(x0f, x1f, weights, core) for core in range(8)]
    nc = _get_nc()
    res = run_bass_kernel_spmd(nc, in_maps, core_ids=list(range(8)))

    out = np.empty((B, C, N), dtype=np.float32)
    for core in range(8):
        b, h = core // 2, core % 2
        out[b][:, h * HC:(h + 1) * HC] = res.results[core]["out"]
    return out.reshape(B, C, H, W)
